# revision 8
# baseline (speedup 1.0000x reference)
"""Trainium2 Bass kernel for nn_LlamaAttention_13383118095011.

Contract: kernel(**inputs) takes FULL inputs (B=8, L=680, C=1536) and
returns the FULL output [8, 680, 1536] fp32.

Sharding: pure data parallel — batch b -> NeuronCore b. No collectives.

Per-core device program (all matmuls bf16 with fp32 PSUM accumulation):
  phase 1: V = X @ Wv^T in [t, f] layout (with a ones column per head for
           the softmax denominator), then QK^T = Wqk @ X^T in [f, l]
           layout; RoPE applied on VectorE in the transposed layout.
  phase 2: per head h: S^T[t,s] = K'^T.T @ Q'^T on PE; P^T = exp(S^T/8)
           on ScalarE (scale folded into the activation); O^T_aug =
           V_aug^T @ P^T on PE (row 64 = softmax denominator D[s]);
           normalize O^T with a K=1 broadcast matmul of 1/D.
  phase 3: Y^T = Wo @ O^T + b_eff, DMA out; host transposes back.

Host-side prep: transposes/casts of x and the weights to bf16 (layout
only), RoPE cos/sin tables (rotate-half sign folded into sin), and
b_eff = Wo @ v_bias + b_o — softmax rows sum to 1, so v_bias passes
through attention unchanged and folds into the output bias exactly.
"""

import sys

if "/opt/trn_rl_repo" not in sys.path:
    sys.path.insert(0, "/opt/trn_rl_repo")

import ml_dtypes
import numpy as np

B = 8
L = 680
C = 1536
H = 24
HD = 64
NCHUNK = C // 128          # 12 contraction chunks
NQK = 2 * C // 128         # 24 f-tiles for fused Q,K
T_SIZES = [128, 128, 128, 128, 128, 40]   # 680 = 5*128 + 40
NS = [(0, 512), (512, 168)]               # free-dim splits (PSUM bank = 512 fp32)
BF = ml_dtypes.bfloat16

_CACHE: dict = {}


def _emit(tc, io):
    import concourse.mybir as mybir

    nc = tc.nc
    f32 = mybir.dt.float32
    bf16 = mybir.dt.bfloat16
    Copy = mybir.ActivationFunctionType.Copy
    Ident = mybir.ActivationFunctionType.Identity
    Exp = mybir.ActivationFunctionType.Exp

    xT, wqkT, wvT, woT, cosT, sinT, qb, beff, yT = (
        io["xT"], io["wqkT"], io["wvT"], io["woT"], io["cosT"], io["sinT"],
        io["qb"], io["beff"], io["yT"],
    )

    const = tc.alloc_tile_pool(name="const", bufs=1)
    wpool = tc.alloc_tile_pool(name="w", bufs=1)
    sb = tc.alloc_tile_pool(name="sb", bufs=1)
    qkp = tc.alloc_tile_pool(name="qkp", bufs=6)
    tmp = tc.alloc_tile_pool(name="tmp", bufs=2)
    ptp = tc.alloc_tile_pool(name="pt", bufs=7)
    bc = tc.alloc_tile_pool(name="bc", bufs=2)
    ps_big = tc.alloc_tile_pool(name="ps_big", bufs=3, space="PSUM")
    ps_bc = tc.alloc_tile_pool(name="ps_bc", bufs=1, space="PSUM")
    pools = [const, wpool, sb, qkp, tmp, ptp, bc, ps_big, ps_bc]

    # ---- constants ----
    cos_sb = const.tile([128, L], bf16, tag="cos")
    sin_sb = const.tile([128, L], bf16, tag="sin")
    qb_sb = const.tile([128, 12], f32, tag="qb")
    beff_sb = const.tile([128, 12], f32, tag="beff")
    ones_sb = const.tile([1, 64], bf16, tag="ones")
    nc.sync.dma_start(cos_sb[:], cosT)
    nc.sync.dma_start(sin_sb[:], sinT)
    nc.sync.dma_start(qb_sb[:], qb)
    nc.sync.dma_start(beff_sb[:], beff)
    nc.vector.memset(ones_sb[:], 1.0)

    # ---- loads: x chunks, then Wv (phase-1a deps), then Wqk ----
    xt = []
    for k in range(NCHUNK):
        t = wpool.tile([128, L], bf16, tag=f"xt{k}", name=f"xt{k}")
        nc.sync.dma_start(t[:], xT[k * 128:(k + 1) * 128, :])
        xt.append(t)
    # wv and wo share the same slots ("ws"): wo_k's DMA starts once wv_k dies
    wv = []
    for k in range(NCHUNK):
        t = wpool.tile([128, C], bf16, tag="ws", name=f"wv{k}", bufs=NCHUNK)
        nc.sync.dma_start(t[:], wvT[k * 128:(k + 1) * 128, :])
        wv.append(t)
    wqk = []
    for k in range(NCHUNK):
        t = wpool.tile([128, 2 * C], bf16, tag=f"wqk{k}", name=f"wqk{k}")
        nc.sync.dma_start(t[:], wqkT[k * 128:(k + 1) * 128, :])
        wqk.append(t)

    # ---- phase 1a: V in [t, f] layout with ones column per head ----
    # va tile free layout: head h occupies cols [h*65, h*65+64), col h*65+64 = 1.0
    va = []
    for ti, tsz in enumerate(T_SIZES):
        t0 = ti * 128
        vt = sb.tile([128, H * 65], bf16, tag=f"va{ti}", name=f"va{ti}")
        vt3 = vt[:].rearrange("p (h e) -> p h e", e=65)
        for fb in range(3):  # 1536 = 3 * 512
            ps = ps_big.tile([128, L], f32, tag="ps", name=f"vps{ti}_{fb}")
            for k in range(NCHUNK):
                nc.tensor.matmul(
                    ps[:tsz, 0:512],
                    xt[k][:, t0:t0 + tsz],
                    wv[k][:, fb * 512:(fb + 1) * 512],
                    start=(k == 0),
                    stop=(k == NCHUNK - 1),
                )
            ps3 = ps[:tsz, 0:512].rearrange("p (h e) -> p h e", e=64)
            nc.vector.tensor_copy(vt3[:tsz, fb * 8:(fb + 1) * 8, 0:64], ps3)
        nc.vector.memset(vt3[:tsz, :, 64:65], 1.0)
        va.append(vt)

    # ---- Wo prefetch: slots free up as phase 1a consumes wv ----
    wo = []
    for k in range(NCHUNK):
        t = wpool.tile([128, C], bf16, tag="ws", name=f"wo{k}", bufs=NCHUNK)
        nc.sync.dma_start(t[:], woT[k * 128:(k + 1) * 128, :])
        wo.append(t)

    # ---- interleaved: per j, project+RoPE Q_j and K_j, then heads 2j, 2j+1 ----
    ot = [sb.tile([128, L], bf16, tag=f"ot{j}", name=f"ot{j}") for j in range(NCHUNK)]
    nt = len(T_SIZES)

    def qk_project(j):
        """QKV^T f-tile j (j<12: Q tile, else K tile) + RoPE -> bf16 SBUF."""
        ps = ps_big.tile([128, L], f32, tag="ps", name=f"qkps{j}")
        for k in range(NCHUNK):
            lhsT = wqk[k][:, j * 128:(j + 1) * 128]
            for (n0, nsz) in NS:
                nc.tensor.matmul(
                    ps[:, n0:n0 + nsz],
                    lhsT,
                    xt[k][:, n0:n0 + nsz],
                    start=(k == 0),
                    stop=(k == NCHUNK - 1),
                )
        qsb = tmp.tile([128, L], bf16, tag="qsb", name=f"qsb{j}")
        if j < NCHUNK:  # q bias (k bias is fixed zero)
            nc.scalar.activation(qsb[:], ps[:], Ident, bias=qb_sb[:, j:j + 1])
        else:
            nc.scalar.activation(qsb[:], ps[:], Copy)
        # RoPE: q' = q*cos + shuffle(q)*sin_signed  (two heads per tile)
        tcos = tmp.tile([128, L], bf16, tag="tcos", name=f"tcos{j}")
        tsin = tmp.tile([128, L], bf16, tag="tsin", name=f"tsin{j}")
        nc.vector.tensor_mul(tcos[:], qsb[:], cos_sb[:])
        for (d0, s0) in ((0, 32), (32, 0), (64, 96), (96, 64)):
            nc.vector.tensor_mul(
                tsin[d0:d0 + 32, :], qsb[s0:s0 + 32, :], sin_sb[s0:s0 + 32, :]
            )
        qf = qkp.tile([128, L], bf16, tag="qk", name=f"qk{j}")
        nc.vector.tensor_add(qf[:], tcos[:], tsin[:])
        return qf

    for j in range(NCHUNK):
        qt = qk_project(j)
        kt = qk_project(NCHUNK + j)
        for h in (2 * j, 2 * j + 1):
            r0 = (h % 2) * 64
            # scores S^T[t, s] then P^T = exp(S^T / 8)
            pts = []
            for ti, tsz in enumerate(T_SIZES):
                t0 = ti * 128
                sps = ps_big.tile([128, L], f32, tag="ps", name=f"sps{h}_{ti}")
                lhsT = kt[r0:r0 + 64, t0:t0 + tsz]
                for (n0, nsz) in NS:
                    nc.tensor.matmul(
                        sps[:tsz, n0:n0 + nsz],
                        lhsT,
                        qt[r0:r0 + 64, n0:n0 + nsz],
                        start=True,
                        stop=True,
                    )
                pt = ptp.tile([128, L], bf16, tag="pt", name=f"pt{h}_{ti}")
                nc.scalar.activation(pt[:tsz, :], sps[:tsz, :], Exp, scale=0.125)
                pts.append(pt)
            # O^T_aug = V_aug^T @ P^T  (row 64 = denominator D[s])
            pv = ps_big.tile([128, L], f32, tag="ps", name=f"pv{h}")
            for ti, tsz in enumerate(T_SIZES):
                lhsT = va[ti][:tsz, h * 65:(h + 1) * 65]
                for (n0, nsz) in NS:
                    nc.tensor.matmul(
                        pv[0:65, n0:n0 + nsz],
                        lhsT,
                        pts[ti][:tsz, n0:n0 + nsz],
                        start=(ti == 0),
                        stop=(ti == nt - 1),
                    )
            # normalize: O^T[d, s] * (1/D[s]) via K=1 broadcast matmul
            rd = bc.tile([1, L], bf16, tag="rd", name=f"rd{h}")
            nc.vector.reciprocal(rd[:], pv[64:65, :])
            bps = ps_bc.tile([64, L], f32, tag="bps", name=f"bps{h}")
            for (n0, nsz) in NS:
                nc.tensor.matmul(
                    bps[:, n0:n0 + nsz],
                    ones_sb[0:1, :],
                    rd[0:1, n0:n0 + nsz],
                    start=True,
                    stop=True,
                )
            bsb = bc.tile([64, L], bf16, tag="bsb", name=f"bsb{h}")
            nc.vector.tensor_copy(bsb[:], bps[:])
            ro = (h % 2) * 64
            nc.vector.tensor_mul(ot[h // 2][ro:ro + 64, :], pv[0:64, :], bsb[:])

    # ---- phase 3: output projection Y^T = Wo @ O^T + beff ----
    for j in range(NCHUNK):
        ps = ps_big.tile([128, L], f32, tag="ps", name=f"yps{j}")
        for k in range(NCHUNK):
            lhsT = wo[k][:, j * 128:(j + 1) * 128]
            for (n0, nsz) in NS:
                nc.tensor.matmul(
                    ps[:, n0:n0 + nsz],
                    lhsT,
                    ot[k][:, n0:n0 + nsz],
                    start=(k == 0),
                    stop=(k == NCHUNK - 1),
                )
        ysb = tmp.tile([128, L], f32, tag="ysb", name=f"ysb{j}")
        nc.scalar.activation(ysb[:], ps[:], Ident, bias=beff_sb[:, j:j + 1])
        nc.sync.dma_start(yT[j * 128:(j + 1) * 128, :], ysb[:])

    for p in reversed(pools):
        p.release()


def build_module():
    import concourse.bacc as bacc
    import concourse.mybir as mybir
    import concourse.tile as tile

    nc = bacc.Bacc("TRN2", target_bir_lowering=False, debug=False)
    f32 = mybir.dt.float32
    bf16 = mybir.dt.bfloat16
    io = {
        "xT": nc.dram_tensor("xT", [C, L], bf16, kind="ExternalInput").ap(),
        "wqkT": nc.dram_tensor("wqkT", [C, 2 * C], bf16, kind="ExternalInput").ap(),
        "wvT": nc.dram_tensor("wvT", [C, C], bf16, kind="ExternalInput").ap(),
        "woT": nc.dram_tensor("woT", [C, C], bf16, kind="ExternalInput").ap(),
        "cosT": nc.dram_tensor("cosT", [128, L], bf16, kind="ExternalInput").ap(),
        "sinT": nc.dram_tensor("sinT", [128, L], bf16, kind="ExternalInput").ap(),
        "qb": nc.dram_tensor("qb", [128, 12], f32, kind="ExternalInput").ap(),
        "beff": nc.dram_tensor("beff", [128, 12], f32, kind="ExternalInput").ap(),
        "yT": nc.dram_tensor("yT", [C, L], f32, kind="ExternalOutput").ap(),
    }
    with nc.allow_low_precision("bf16 softmax denominator for broadcast matmul"):
        with tile.TileContext(nc) as tc:
            _emit(tc, io)
    nc.compile()
    return nc


def get_module():
    if "nc" not in _CACHE:
        _CACHE["nc"] = build_module()
    return _CACHE["nc"]


def host_prep(inputs):
    """Shard + lay out inputs for the 8 cores. Returns in_maps."""
    x = np.asarray(inputs["x"], np.float32)
    pos = np.asarray(inputs["pos_ids"], np.float32)
    wqkv = np.asarray(inputs["w_qkv"], np.float32)
    q_bias = np.asarray(inputs["q_bias"], np.float32)
    v_bias = np.asarray(inputs["v_bias"], np.float32)
    w_o = np.asarray(inputs["w_o"], np.float32)
    b_o = np.asarray(inputs["b_o"], np.float32)

    wqkT = np.ascontiguousarray(wqkv[: 2 * C].T).astype(BF)
    wvT = np.ascontiguousarray(wqkv[2 * C:].T).astype(BF)
    woT = np.ascontiguousarray(w_o.T).astype(BF)
    beff = w_o @ v_bias + b_o
    qb_t = np.ascontiguousarray(q_bias.reshape(12, 128).T, dtype=np.float32)
    beff_t = np.ascontiguousarray(beff.reshape(12, 128).T, dtype=np.float32)

    inv = 1.0 / (10000.0 ** (np.arange(0, HD, 4, dtype=np.float32) / HD))
    sgn = np.where(np.arange(HD) % HD < 32, -1.0, 1.0).astype(np.float32)

    in_maps = []
    for b in range(B):
        f = pos[b][:, :, None] * inv                       # [L, 2, 16]
        emb = np.concatenate([f, f], -1).reshape(L, HD)    # [L, 64]
        cos = np.cos(emb)
        sin2 = np.sin(emb) * sgn
        cosT = np.ascontiguousarray(np.tile(cos.T, (2, 1))).astype(BF)
        s2 = np.tile(sin2.T, (2, 1))   # [128, L]
        sinT = np.ascontiguousarray(
            np.vstack([s2[32:64], s2[0:32], s2[96:128], s2[64:96]])
        ).astype(BF)
        xT = np.ascontiguousarray(x[b].T).astype(BF)
        in_maps.append({
            "xT": xT, "wqkT": wqkT, "wvT": wvT, "woT": woT,
            "cosT": cosT, "sinT": sinT, "qb": qb_t, "beff": beff_t,
        })
    return in_maps


def kernel(**inputs):
    from concourse.bass_utils import run_bass_kernel_spmd

    nc = get_module()
    in_maps = host_prep(inputs)
    res = run_bass_kernel_spmd(nc, in_maps, list(range(B))).results
    y = np.stack([np.asarray(res[b]["yT"], np.float32).T for b in range(B)])
    return np.ascontiguousarray(y)


# revision 12
# speedup vs baseline: 1.4084x; 1.4084x over previous
"""Trainium2 Bass kernel for nn_LlamaAttention_13383118095011.

Contract: kernel(**inputs) takes FULL inputs (B=8, L=680, C=1536) and
returns the FULL output [8, 680, 1536] fp32.

Sharding: pure data parallel — batch b -> NeuronCore b. No collectives.

Per-core device program (all matmuls bf16 with fp32 PSUM accumulation):
  phase 1: V = X @ Wv^T in [t, f] layout (with a ones column per head for
           the softmax denominator), then QK^T = Wqk @ X^T in [f, l]
           layout; RoPE applied on VectorE in the transposed layout.
  phase 2: per head h: S^T[t,s] = K'^T.T @ Q'^T on PE; P^T = exp(S^T/8)
           on ScalarE (scale folded into the activation); O^T_aug =
           V_aug^T @ P^T on PE (row 64 = softmax denominator D[s]);
           normalize O^T with a K=1 broadcast matmul of 1/D.
  phase 3: Y^T = Wo @ O^T + b_eff, DMA out; host transposes back.

Host-side prep: transposes/casts of x and the weights to bf16 (layout
only), RoPE cos/sin tables (rotate-half sign folded into sin), and
b_eff = Wo @ v_bias + b_o — softmax rows sum to 1, so v_bias passes
through attention unchanged and folds into the output bias exactly.
"""

import sys

if "/opt/trn_rl_repo" not in sys.path:
    sys.path.insert(0, "/opt/trn_rl_repo")

import ml_dtypes
import numpy as np

B = 8
L = 680
C = 1536
H = 24
HD = 64
NCHUNK = C // 128          # 12 contraction chunks
NQK = 2 * C // 128         # 24 f-tiles for fused Q,K
T_SIZES = [128, 128, 128, 128, 128, 40]   # 680 = 5*128 + 40
NS = [(0, 512), (512, 168)]               # free-dim splits (PSUM bank = 512 fp32)
BF = ml_dtypes.bfloat16

_CACHE: dict = {}


def _emit(tc, io):
    import concourse.mybir as mybir

    nc = tc.nc
    f32 = mybir.dt.float32
    bf16 = mybir.dt.bfloat16
    Copy = mybir.ActivationFunctionType.Copy
    Ident = mybir.ActivationFunctionType.Identity
    Exp = mybir.ActivationFunctionType.Exp

    xT, wqkT, wvT, woT, cosT, sinT, qb, beff, yT = (
        io["xT"], io["wqkT"], io["wvT"], io["woT"], io["cosT"], io["sinT"],
        io["qb"], io["beff"], io["yT"],
    )

    const = tc.alloc_tile_pool(name="const", bufs=1)
    wpool = tc.alloc_tile_pool(name="w", bufs=1)
    sb = tc.alloc_tile_pool(name="sb", bufs=1)
    qkp = tc.alloc_tile_pool(name="qkp", bufs=6)
    tmp = tc.alloc_tile_pool(name="tmp", bufs=2)
    ptp = tc.alloc_tile_pool(name="pt", bufs=7)
    bc = tc.alloc_tile_pool(name="bc", bufs=2)
    ps_qkv = tc.alloc_tile_pool(name="ps_qkv", bufs=1, space="PSUM")
    ps_att = tc.alloc_tile_pool(name="ps_att", bufs=2, space="PSUM")
    ps_pv = tc.alloc_tile_pool(name="ps_pv", bufs=1, space="PSUM")
    pools = [const, wpool, sb, qkp, tmp, ptp, bc, ps_qkv, ps_att, ps_pv]


    # ---- constants ----
    cos_sb = const.tile([128, L], bf16, tag="cos")
    sin_sb = const.tile([128, L], bf16, tag="sin")
    qb_sb = const.tile([128, 12], f32, tag="qb")
    beff_sb = const.tile([128, 12], f32, tag="beff")
    ones_sb = const.tile([1, 64], bf16, tag="ones")
    nc.sync.dma_start(cos_sb[:], cosT)
    nc.sync.dma_start(sin_sb[:], sinT)
    nc.sync.dma_start(qb_sb[:], qb)
    nc.sync.dma_start(beff_sb[:], beff)
    nc.vector.memset(ones_sb[:], 1.0)

    # ---- loads: x chunks, then Wv (phase-1a deps), then Wqk ----
    # wv and wo share the same slots ("ws"): wo_k's DMA starts once wv_k dies
    xt, wv = [], []
    for k in range(NCHUNK):
        t = wpool.tile([128, L], bf16, tag=f"xt{k}", name=f"xt{k}")
        nc.sync.dma_start(t[:], xT[k * 128:(k + 1) * 128, :])
        xt.append(t)
        t = wpool.tile([128, C], bf16, tag="ws", name=f"wv{k}", bufs=NCHUNK)
        nc.sync.dma_start(t[:], wvT[k * 128:(k + 1) * 128, :])
        wv.append(t)
    wqk = []
    for k in range(NCHUNK):
        t = wpool.tile([128, 2 * C], bf16, tag=f"wqk{k}", name=f"wqk{k}")
        nc.sync.dma_start(t[:], wqkT[k * 128:(k + 1) * 128, :])
        wqk.append(t)

    # ---- phase 1a: V in [t, f] layout with ones column per head ----
    # va tile free layout: head h occupies cols [h*65, h*65+64), col h*65+64 = 1.0
    va = []
    for ti, tsz in enumerate(T_SIZES):
        t0 = ti * 128
        vt = sb.tile([128, H * 65], bf16, tag=f"va{ti}", name=f"va{ti}")
        vt3 = vt[:].rearrange("p (h e) -> p h e", e=65)
        for fb in range(3):  # 1536 = 3 * 512
            ps = ps_att.tile([128, L], f32, tag="ps", name=f"vps{ti}_{fb}")
            for k in range(NCHUNK):
                nc.tensor.matmul(
                    ps[:tsz, 0:512],
                    xt[k][:, t0:t0 + tsz],
                    wv[k][:, fb * 512:(fb + 1) * 512],
                    start=(k == 0),
                    stop=(k == NCHUNK - 1),
                )
            ps3 = ps[:tsz, 0:512].rearrange("p (h e) -> p h e", e=64)
            nc.vector.tensor_copy(vt3[:tsz, fb * 8:(fb + 1) * 8, 0:64], ps3)
        nc.vector.memset(vt3[:tsz, :, 64:65], 1.0)
        va.append(vt)

    # ---- Wo prefetch: slots free up as phase 1a consumes wv ----
    wo = []
    for k in range(NCHUNK):
        t = wpool.tile([128, C], bf16, tag="ws", name=f"wo{k}", bufs=NCHUNK)
        nc.sync.dma_start(t[:], woT[k * 128:(k + 1) * 128, :])
        wo.append(t)

    # ---- interleaved: per j, project+RoPE Q_j and K_j, then heads 2j, 2j+1 ----
    ot = [sb.tile([128, L], bf16, tag=f"ot{j}", name=f"ot{j}") for j in range(NCHUNK)]
    nt = len(T_SIZES)

    def qk_project(j):
        """QKV^T f-tile j (j<12: Q tile, else K tile) + RoPE -> bf16 SBUF."""
        ps = ps_qkv.tile([128, L], f32, tag="psq", name=f"qkps{j}")
        for k in range(NCHUNK):
            lhsT = wqk[k][:, j * 128:(j + 1) * 128]
            for (n0, nsz) in NS:
                nc.tensor.matmul(
                    ps[:, n0:n0 + nsz],
                    lhsT,
                    xt[k][:, n0:n0 + nsz],
                    start=(k == 0),
                    stop=(k == NCHUNK - 1),
                )
        qsb = tmp.tile([128, L], bf16, tag="qsb", name=f"qsb{j}")
        if j < NCHUNK:  # q bias (k bias is fixed zero)
            nc.scalar.activation(qsb[:], ps[:], Ident, bias=qb_sb[:, j:j + 1])
        else:
            nc.scalar.activation(qsb[:], ps[:], Copy)
        # RoPE: q' = q*cos + shuffle(q)*sin_signed  (two heads per tile)
        tcos = tmp.tile([128, L], bf16, tag="tcos", name=f"tcos{j}")
        tsin = tmp.tile([128, L], bf16, tag="tsin", name=f"tsin{j}")
        nc.vector.tensor_mul(tcos[:], qsb[:], cos_sb[:])
        for (d0, s0) in ((0, 32), (32, 0), (64, 96), (96, 64)):
            nc.vector.tensor_mul(
                tsin[d0:d0 + 32, :], qsb[s0:s0 + 32, :], sin_sb[s0:s0 + 32, :]
            )
        qf = qkp.tile([128, L], bf16, tag="qk", name=f"qk{j}")
        nc.vector.tensor_add(qf[:], tcos[:], tsin[:])
        return qf

    def attention_head(h, qt, kt):
        r0 = (h % 2) * 64
        # scores S^T[t, s] then P^T = exp(S^T / 8)
        pts = []
        for ti, tsz in enumerate(T_SIZES):
            t0 = ti * 128
            sps = ps_att.tile([128, L], f32, tag="ps", name=f"sps{h}_{ti}")
            lhsT = kt[r0:r0 + 64, t0:t0 + tsz]
            for (n0, nsz) in NS:
                nc.tensor.matmul(
                    sps[:tsz, n0:n0 + nsz],
                    lhsT,
                    qt[r0:r0 + 64, n0:n0 + nsz],
                    start=True,
                    stop=True,
                )
            pt = ptp.tile([128, L], bf16, tag="pt", name=f"pt{h}_{ti}")
            nc.scalar.activation(pt[:tsz, :], sps[:tsz, :], Exp, scale=0.125)
            pts.append(pt)
        # O^T_aug = V_aug^T @ P^T  (row 64 = denominator D[s])
        pv = ps_pv.tile([128, L], f32, tag="psv", name=f"pv{h}")
        for ti, tsz in enumerate(T_SIZES):
            lhsT = va[ti][:tsz, h * 65:(h + 1) * 65]
            for (n0, nsz) in NS:
                nc.tensor.matmul(
                    pv[0:65, n0:n0 + nsz],
                    lhsT,
                    pts[ti][:tsz, n0:n0 + nsz],
                    start=(ti == 0),
                    stop=(ti == len(T_SIZES) - 1),
                )
        # normalize: O^T[d, s] * (1/D[s]) via K=1 bcast matmul
        dsb = bc.tile([1, L], f32, tag="dsb", name=f"dsb{h}")
        nc.vector.tensor_copy(dsb[:], pv[64:65, :])
        rdb = bc.tile([1, L], bf16, tag="rdb", name=f"rdb{h}")
        nc.vector.reciprocal_approx_fast(dsb[:], dsb[:])
        nc.vector.tensor_copy(rdb[:], dsb[:])
        bps = ps_att.tile([128, L], f32, tag="ps", name=f"bps{h}")
        for (n0, nsz) in NS:
            nc.tensor.matmul(
                bps[0:64, n0:n0 + nsz],
                ones_sb[0:1, :],
                rdb[0:1, n0:n0 + nsz],
                start=True,
                stop=True,
            )
        bsb = bc.tile([64, L], bf16, tag="bsb", name=f"bsb{h}")
        nc.scalar.activation(bsb[:], bps[0:64, :], Copy)
        ro = (h % 2) * 64
        nc.vector.tensor_mul(ot[h // 2][ro:ro + 64, :], pv[0:64, :], bsb[:])

    import os
    if os.environ.get("K_NO_PIPELINE"):
        for j in range(NCHUNK):
            qt = qk_project(j)
            kt = qk_project(NCHUNK + j)
            attention_head(2 * j, qt, kt)
            attention_head(2 * j + 1, qt, kt)
    else:
        prev = None
        for j in range(NCHUNK):
            qt = qk_project(j)
            kt = qk_project(NCHUNK + j)
            if prev is not None:
                attention_head(2 * prev[2], prev[0], prev[1])
                attention_head(2 * prev[2] + 1, prev[0], prev[1])
            prev = (qt, kt, j)
        attention_head(2 * prev[2], prev[0], prev[1])
        attention_head(2 * prev[2] + 1, prev[0], prev[1])

    # ---- phase 3: output projection Y^T = Wo @ O^T + beff ----
    for j in range(NCHUNK):
        ps = ps_att.tile([128, L], f32, tag="ps", name=f"yps{j}")
        for k in range(NCHUNK):
            lhsT = wo[k][:, j * 128:(j + 1) * 128]
            for (n0, nsz) in NS:
                nc.tensor.matmul(
                    ps[:, n0:n0 + nsz],
                    lhsT,
                    ot[k][:, n0:n0 + nsz],
                    start=(k == 0),
                    stop=(k == NCHUNK - 1),
                )
        ysb = tmp.tile([128, L], f32, tag="ysb", name=f"ysb{j}")
        nc.scalar.activation(ysb[:], ps[:], Ident, bias=beff_sb[:, j:j + 1])
        nc.sync.dma_start(yT[j * 128:(j + 1) * 128, :], ysb[:])

    for p in reversed(pools):
        p.release()


def build_module():
    import concourse.bacc as bacc
    import concourse.mybir as mybir
    import concourse.tile as tile

    nc = bacc.Bacc("TRN2", target_bir_lowering=False, debug=False)
    f32 = mybir.dt.float32
    bf16 = mybir.dt.bfloat16
    io = {
        "xT": nc.dram_tensor("xT", [C, L], bf16, kind="ExternalInput").ap(),
        "wqkT": nc.dram_tensor("wqkT", [C, 2 * C], bf16, kind="ExternalInput").ap(),
        "wvT": nc.dram_tensor("wvT", [C, C], bf16, kind="ExternalInput").ap(),
        "woT": nc.dram_tensor("woT", [C, C], bf16, kind="ExternalInput").ap(),
        "cosT": nc.dram_tensor("cosT", [128, L], bf16, kind="ExternalInput").ap(),
        "sinT": nc.dram_tensor("sinT", [128, L], bf16, kind="ExternalInput").ap(),
        "qb": nc.dram_tensor("qb", [128, 12], f32, kind="ExternalInput").ap(),
        "beff": nc.dram_tensor("beff", [128, 12], f32, kind="ExternalInput").ap(),
        "yT": nc.dram_tensor("yT", [C, L], f32, kind="ExternalOutput").ap(),
    }
    with nc.allow_low_precision("bf16 softmax denominator for broadcast matmul"):
        with tile.TileContext(nc) as tc:
            _emit(tc, io)
    nc.compile()
    return nc


def get_module():
    if "nc" not in _CACHE:
        _CACHE["nc"] = build_module()
    return _CACHE["nc"]


def host_prep(inputs):
    """Shard + lay out inputs for the 8 cores. Returns in_maps."""
    x = np.asarray(inputs["x"], np.float32)
    pos = np.asarray(inputs["pos_ids"], np.float32)
    wqkv = np.asarray(inputs["w_qkv"], np.float32)
    q_bias = np.asarray(inputs["q_bias"], np.float32)
    v_bias = np.asarray(inputs["v_bias"], np.float32)
    w_o = np.asarray(inputs["w_o"], np.float32)
    b_o = np.asarray(inputs["b_o"], np.float32)

    wqkT = np.ascontiguousarray(wqkv[: 2 * C].T).astype(BF)
    wvT = np.ascontiguousarray(wqkv[2 * C:].T).astype(BF)
    woT = np.ascontiguousarray(w_o.T).astype(BF)
    beff = w_o @ v_bias + b_o
    qb_t = np.ascontiguousarray(q_bias.reshape(12, 128).T, dtype=np.float32)
    beff_t = np.ascontiguousarray(beff.reshape(12, 128).T, dtype=np.float32)

    inv = 1.0 / (10000.0 ** (np.arange(0, HD, 4, dtype=np.float32) / HD))
    sgn = np.where(np.arange(HD) % HD < 32, -1.0, 1.0).astype(np.float32)

    in_maps = []
    for b in range(B):
        f = pos[b][:, :, None] * inv                       # [L, 2, 16]
        emb = np.concatenate([f, f], -1).reshape(L, HD)    # [L, 64]
        cos = np.cos(emb)
        sin2 = np.sin(emb) * sgn
        cosT = np.ascontiguousarray(np.tile(cos.T, (2, 1))).astype(BF)
        s2 = np.tile(sin2.T, (2, 1))   # [128, L]
        sinT = np.ascontiguousarray(
            np.vstack([s2[32:64], s2[0:32], s2[96:128], s2[64:96]])
        ).astype(BF)
        xT = np.ascontiguousarray(x[b].T).astype(BF)
        in_maps.append({
            "xT": xT, "wqkT": wqkT, "wvT": wvT, "woT": woT,
            "cosT": cosT, "sinT": sinT, "qb": qb_t, "beff": beff_t,
        })
    return in_maps


def kernel(**inputs):
    from concourse.bass_utils import run_bass_kernel_spmd

    nc = get_module()
    in_maps = host_prep(inputs)
    res = run_bass_kernel_spmd(nc, in_maps, list(range(B))).results
    y = np.stack([np.asarray(res[b]["yT"], np.float32).T for b in range(B)])
    return np.ascontiguousarray(y)
